# revision 17
# baseline (speedup 1.0000x reference)
"""AugmentedTripletLoss kernel for 8 Trainium2 NeuronCores.

Strategy (data-parallel over rows, per sharding hint) — v4:
  - Rows AND columns are globally sorted by class (host-side, free: the
    loss is a mean over rows, permutation-invariant).  Core k takes
    sorted rows [1024k, 1024k+1024) and sees the 8192 columns ROTATED
    by 1024k, so its own rows sit at columns [0, 1024).  Same-class
    columns for m-tile m then live in the fixed strip
    [128m-STRIP, 128m+128+STRIP) mod 8192 (valid while every class has
    <= STRIP members; multinomial(8192, 64) gives ~128 +- 11, max ~165).
  - Per m-tile the [128, 8208] block of
        D(i,j) = dist2(i,j) - sq_i + BIG*mask(i,j)
    is built with two accumulated bf16 matmul passes per [128,1024]
    PSUM tile, with the two passes grouped across block pairs so
    LDWEIGHTS swaps half as often.  All operands are pre-baked on the
    host; input DMAs are spread over four engine queues so the first
    matmul starts ~2us in, and a burst of garbage-fed warmup matmuls
    un-throttles the PE clock (HAM) while they land.
  - Mining per m-tile:
      dist_ap^2: tensor_reduce(max) over the same-class strip read
        directly from PSUM in fp32 (+BIG selects same-class there).
      dist_an^2: blocks 0-5 are copied to SBUF as bf16 by the Scalar
        engine and min-combined by a DVE tensor_tensor tree (2x bf16
        mode) with GpSimd taking two interior tree ops; blocks 6-7 are
        min-reduced straight from PSUM by the DVE.  The +BIG mask keeps
        same-class out of every min path; centers join via one small
        reduce.
  - Epilogue: sqrt on Scalar (table preloaded at t=0), relu on DVE,
    row-sum via a ones-matmul; per-core partials are averaged on the
    host (the "all-reduce mean").
"""

import numpy as np

N, D, NCTR, C = 8192, 128, 16, 64
NCORES = 8
RPC = N // NCORES          # rows per core = 1024
MT = RPC // 128            # m-tiles per core = 8
NCOL = N + NCTR            # 8208 columns (samples + centers)
NBLK = 8                   # full [128,1024] column blocks per m-tile
BIG = 4096.0
S = 64.0                   # sqrt(BIG)
MARGIN = 1.0
EPS = 1e-12
STRIP = 192                # strip margin (max class size it tolerates)
NCOPY = 6                  # blocks per m-tile copied to SBUF (rest: direct)

_CACHE = {}


def _strip_slices(m):
    """Per m-tile: 1-2 (block, lo, hi) slices covering the same-class strip
    [128*m - STRIP, 128*m + 128 + STRIP) in rotated column space."""
    lo = 128 * m - STRIP
    hi = 128 * m + 128 + STRIP
    out = []
    if lo < 0:
        out.append((7, 1024 + lo, 1024))
        lo = 0
    if hi <= 1024:
        out.append((0, lo, hi))
    else:
        out.append((0, lo, 1024))
        out.append((1, 0, hi - 1024))
    return out


def _build_program(warm_mms=8):
    from concourse import bacc, mybir, tile
    from concourse.bass import ts

    f32 = mybir.dt.float32
    bf16 = mybir.dt.bfloat16
    X = mybir.AxisListType.X
    Alu = mybir.AluOpType

    nc = bacc.Bacc(
        "TRN2", target_bir_lowering=False, debug=False, enable_asserts=False
    )

    rhs1_d = nc.dram_tensor("rhs1", [D, NCOL], bf16, kind="ExternalInput").ap()
    rhs2_d = nc.dram_tensor("rhs2", [C + 1, NCOL], bf16, kind="ExternalInput").ap()
    lhs1_d = nc.dram_tensor("lhs1", [D, RPC], bf16, kind="ExternalInput").ap()
    lhs2_d = nc.dram_tensor("lhs2", [C + 1, RPC], bf16, kind="ExternalInput").ap()
    sqi_d = nc.dram_tensor("sqi", [128, MT], f32, kind="ExternalInput").ap()
    out_d = nc.dram_tensor("out", [1, 1], f32, kind="ExternalOutput").ap()

    with tile.TileContext(nc) as tc:
        with (
            tc.tile_pool(name="per", bufs=1) as per,
            tc.tile_pool(name="cb", bufs=4) as cb,
            tc.tile_pool(name="tp", bufs=6) as tp,
        ):
            # ---- persistent SBUF tensors ----
            rhs1s = per.tile([D, NCOL], bf16, tag="rhs1s")
            rhs2s = per.tile([C + 1, NCOL], bf16, tag="rhs2s")
            lhs1s = per.tile([D, RPC], bf16, tag="lhs1s")
            lhs2s = per.tile([C + 1, RPC], bf16, tag="lhs2s")
            sqi = per.tile([128, MT], f32, tag="sqi")
            mins = per.tile([128, MT * 4], f32, tag="mins")
            maxs = per.tile([128, MT * 2], f32, tag="maxs")
            wgarb = per.tile([128, 512], bf16, tag="wgarb")
            onescol = per.tile([128, 1], f32, tag="onescol")
            sqjunk = per.tile([1, 1], f32, tag="sqjunk")
            outs = per.tile([1, 1], f32, tag="outs")
            pos2 = per.tile([128, MT], f32, tag="pos2")
            neg2 = per.tile([128, MT], f32, tag="neg2")
            apd = per.tile([128, MT], f32, tag="apd")
            andt = per.tile([128, MT], f32, tag="andt")
            rl = per.tile([128, MT], f32, tag="rl")
            rsum = per.tile([128, 1], f32, tag="rsum")

            # ---- input DMAs spread over 4 engine queues, earliest first ----
            # gpsimd leads with the warmup-garbage memset so the PE can
            # start immediately; vector leads with the small memsets.
            nc.gpsimd.memset(wgarb[:, :], 0.0)
            nc.vector.memset(onescol[:, :], 1.0)
            nc.vector.memset(maxs[:, :], -3.0e38)

            def r1(b):
                return (rhs1s[:, 1024 * b : 1024 * b + 1024],
                        rhs1_d[:, 1024 * b : 1024 * b + 1024])

            def r2(b):
                return (rhs2s[:, 1024 * b : 1024 * b + 1024],
                        rhs2_d[:, 1024 * b : 1024 * b + 1024])

            for o, i in [(lhs1s[:, :], lhs1_d[:, :]), r1(0), r2(2),
                         r1(2), r1(4), r1(6)]:
                nc.sync.dma_start(out=o, in_=i)
            for o, i in [(lhs2s[:, :], lhs2_d[:, :]), r1(1), r2(3),
                         r1(3), r1(5), r1(7)]:
                nc.gpsimd.dma_start(out=o, in_=i)
            for o, i in [r2(0), r2(1), r2(4), r2(5), r2(6), r2(7),
                         (rhs1s[:, N:], rhs1_d[:, N:]),
                         (rhs2s[:, N:], rhs2_d[:, N:]),
                         (sqi[:, :], sqi_d[:, :])]:
                nc.scalar.dma_start(out=o, in_=i)
            # preload the sqrt activation table while the sweep runs
            nc.scalar.sqrt(sqjunk[:, :], onescol[0:1, 0:1])

            # ---- main sweep ----
            with tc.tile_pool(name="pp", bufs=4, space="PSUM") as pp:
                if warm_mms:
                    # garbage matmuls: wake HAM out of the throttled clock
                    # while the real inputs are still in flight
                    wpt = pp.tile([128, 1024], f32, tag="ptile")
                    for _ in range(warm_mms):
                        nc.tensor.matmul(
                            wpt[:, 0:512], wgarb[:, 0:128], wgarb[:, :],
                            start=True, stop=True,
                        )

                for m in range(MT):
                    w1 = lhs1s[:, ts(m, 128)]
                    w2 = lhs2s[:, ts(m, 128)]
                    cts = {}          # block -> bf16 SBUF copy
                    tq = tp.tile([128, 3, 1024], bf16, tag="tq")
                    strips = _strip_slices(m)

                    def emit_consumers(b, pt):
                        for si, (blk, lo, hi) in enumerate(strips):
                            if blk == b:
                                nc.vector.tensor_reduce(
                                    maxs[:, 2 * m + si : 2 * m + si + 1],
                                    pt[:, lo:hi],
                                    X,
                                    Alu.max,
                                )
                        if b >= NCOPY:
                            # direct min-reduce from PSUM (fp32)
                            slot = 1 + (b - NCOPY)
                            nc.vector.tensor_reduce(
                                mins[:, 4 * m + slot : 4 * m + slot + 1],
                                pt[:, :],
                                X,
                                Alu.min,
                            )
                            return
                        ct = cb.tile([128, 1024], bf16, tag="ct")
                        nc.scalar.copy(ct[:, :], pt[:, :])
                        cts[b] = ct
                        if b % 2 == 1:
                            nc.vector.tensor_tensor(
                                out=tq[:, b // 2, :], in0=cts[b - 1][:, :],
                                in1=cts[b][:, :], op=Alu.min,
                            )
                            if b == 5:
                                dump = tp.tile([128, 3, 1024], bf16, tag="dump")
                                nc.vector.tensor_scalar(
                                    out=dump[:, :, :],
                                    in0=tq[:, :, :],
                                    scalar1=0.0,
                                    scalar2=None,
                                    op0=Alu.add,
                                    op1=Alu.min,
                                    accum_out=mins[:, 4 * m : 4 * m + 1],
                                )

                    # paired blocks: pass1 over both, then pass2 over both
                    for bp in range(NBLK // 2):
                        b0, b1 = 2 * bp, 2 * bp + 1
                        pta = pp.tile([128, 1024], f32, tag="ptile")
                        ptb = pp.tile([128, 1024], f32, tag="ptile")
                        for pt, b in ((pta, b0), (ptb, b1)):
                            for h in range(2):
                                c0 = 1024 * b + 512 * h
                                nc.tensor.matmul(
                                    pt[:, h * 512 : h * 512 + 512],
                                    w1,
                                    rhs1s[:, c0 : c0 + 512],
                                    start=True,
                                    stop=False,
                                )
                        for pt, b in ((pta, b0), (ptb, b1)):
                            for h in range(2):
                                c0 = 1024 * b + 512 * h
                                nc.tensor.matmul(
                                    pt[:, h * 512 : h * 512 + 512],
                                    w2,
                                    rhs2s[:, c0 : c0 + 512],
                                    start=False,
                                    stop=True,
                                )
                        emit_consumers(b0, pta)
                        emit_consumers(b1, ptb)

                    # centers block (16 cols)
                    ptc = pp.tile([128, 1024], f32, tag="ptile")
                    nc.tensor.matmul(
                        ptc[:, 0:NCTR], w1, rhs1s[:, N:], start=True, stop=False
                    )
                    nc.tensor.matmul(
                        ptc[:, 0:NCTR], w2, rhs2s[:, N:], start=False, stop=True
                    )
                    nc.vector.tensor_reduce(
                        mins[:, 4 * m + 3 : 4 * m + 4], ptc[:, :NCTR], X, Alu.min
                    )

                # ---- epilogue (vectorized over the 8 m-tiles) ----
                posr = per.tile([128, MT], f32, tag="posr")
                negr = per.tile([128, MT], f32, tag="negr")
                nc.vector.tensor_reduce(
                    posr[:, :], maxs[:, :].rearrange("p (t s) -> p t s", s=2),
                    X, Alu.max,
                )
                nc.vector.tensor_reduce(
                    negr[:, :], mins[:, :].rearrange("p (t s) -> p t s", s=4),
                    X, Alu.min,
                )

                nc.vector.tensor_tensor(
                    out=pos2[:, :], in0=posr[:, :], in1=sqi[:, :], op=Alu.add
                )
                nc.vector.tensor_scalar(
                    out=pos2[:, :], in0=pos2[:, :], scalar1=BIG, scalar2=EPS,
                    op0=Alu.subtract, op1=Alu.max,
                )
                nc.scalar.sqrt(apd[:, :], pos2[:, :])

                nc.vector.tensor_tensor(
                    out=neg2[:, :], in0=negr[:, :], in1=sqi[:, :], op=Alu.add
                )
                nc.vector.tensor_scalar(
                    out=neg2[:, :], in0=neg2[:, :], scalar1=EPS, scalar2=None,
                    op0=Alu.max,
                )
                nc.scalar.sqrt(andt[:, :], neg2[:, :])

                nc.vector.tensor_tensor(
                    out=rl[:, :], in0=apd[:, :], in1=andt[:, :], op=Alu.subtract
                )
                nc.vector.tensor_scalar(
                    out=rl[:, :], in0=rl[:, :], scalar1=MARGIN, scalar2=0.0,
                    op0=Alu.add, op1=Alu.max,
                )
                nc.vector.tensor_reduce(rsum[:, :], rl[:, :], X, Alu.add)

                fin = pp.tile([128, 1024], f32, tag="ptile")
                nc.tensor.matmul(
                    fin[0:1, 0:1], onescol[:, :], rsum[:, :], start=True, stop=True
                )
                nc.scalar.copy(outs[:, :], fin[0:1, 0:1])
                nc.sync.dma_start(out=out_d[:, :], in_=outs[:, :])

    nc.compile()
    return nc


def _make_in_maps(inputs, targets, center):
    import ml_dtypes

    bf = ml_dtypes.bfloat16
    x = np.ascontiguousarray(np.asarray(inputs, dtype=np.float32))
    t = np.asarray(targets).astype(np.int64)
    c = np.ascontiguousarray(np.asarray(center, dtype=np.float32))

    # global class sort of rows == columns
    perm = np.argsort(t, kind="stable")
    xs = x[perm]
    ts_ = t[perm]
    sqs = (xs.astype(np.float64) ** 2).sum(1).astype(np.float32)

    cn = c / np.linalg.norm(c, axis=1, keepdims=True)

    xsT = np.ascontiguousarray(xs.T).astype(bf)             # [128, 8192]
    oh = (ts_[None, :] == np.arange(C)[:, None]).astype(np.float32) * S
    ohb = oh.astype(bf)                                      # [64, 8192]
    sq_row = sqs.astype(bf)                                  # [8192]

    in_maps = []
    for k in range(NCORES):
        sh = -RPC * k
        rows = slice(RPC * k, RPC * (k + 1))

        rhs1 = np.empty((D, NCOL), dtype=bf)
        rhs1[:, :N] = np.roll(xsT, sh, axis=1)
        rhs1[:, N:] = cn.T.astype(bf)

        rhs2 = np.zeros((C + 1, NCOL), dtype=bf)
        rhs2[:C, :N] = np.roll(ohb, sh, axis=1)
        rhs2[C, :N] = np.roll(sq_row, sh)
        rhs2[C, N:] = np.ones((NCTR,), dtype=bf)

        lhs1 = np.ascontiguousarray(-2.0 * xsT[:, rows].astype(np.float32)).astype(bf)
        lhs2 = np.concatenate(
            [ohb[:, rows], np.ones((1, RPC), dtype=bf)], axis=0
        )
        sqi = np.ascontiguousarray(
            sqs[rows].reshape(MT, 128).T
        )  # [128, MT]

        in_maps.append(
            {
                "rhs1": rhs1,
                "rhs2": rhs2,
                "lhs1": np.ascontiguousarray(lhs1),
                "lhs2": np.ascontiguousarray(lhs2),
                "sqi": sqi,
            }
        )
    return in_maps


def run(inputs, targets, center, trace=False, tmpdir=None):
    """Returns (loss_scalar, BassKernelResults)."""
    from concourse.bass_utils import run_bass_kernel_spmd

    if "nc" not in _CACHE:
        _CACHE["nc"] = _build_program()
    nc = _CACHE["nc"]
    in_maps = _make_in_maps(inputs, targets, center)
    res = run_bass_kernel_spmd(
        nc, in_maps, list(range(NCORES)), trace=trace, tmpdir=tmpdir
    )
    total = sum(float(r["out"][0, 0]) for r in res.results)
    loss = np.array(total / N, dtype=np.float32)
    return loss, res


def kernel(inputs, targets, center):
    loss, _ = run(inputs, targets, center, trace=False)
    return loss


# revision 18
# speedup vs baseline: 1.3202x; 1.3202x over previous
"""AugmentedTripletLoss kernel for 8 Trainium2 NeuronCores.

Strategy (data-parallel over rows, per sharding hint) — v4:
  - Rows AND columns are globally sorted by class (host-side, free: the
    loss is a mean over rows, permutation-invariant).  Core k takes
    sorted rows [1024k, 1024k+1024) and sees the 8192 columns ROTATED
    by 1024k, so its own rows sit at columns [0, 1024).  Same-class
    columns for m-tile m then live in the fixed strip
    [128m-STRIP, 128m+128+STRIP) mod 8192 (valid while every class has
    <= STRIP members; multinomial(8192, 64) gives ~128 +- 11, max ~165).
  - Per m-tile the [128, 8208] block of
        D(i,j) = dist2(i,j) - sq_i + BIG*mask(i,j)
    is built with two accumulated bf16 matmul passes per [128,1024]
    PSUM tile, with the two passes grouped across block pairs so
    LDWEIGHTS swaps half as often.  All operands are pre-baked on the
    host; input DMAs are spread over four engine queues so the first
    matmul starts ~2us in, and a burst of garbage-fed warmup matmuls
    un-throttles the PE clock (HAM) while they land.
  - Mining per m-tile:
      dist_ap^2: tensor_reduce(max) over the same-class strip read
        directly from PSUM in fp32 (+BIG selects same-class there).
      dist_an^2: blocks 0-5 are copied to SBUF as bf16 by the Scalar
        engine and min-combined by a DVE tensor_tensor tree (2x bf16
        mode) with GpSimd taking two interior tree ops; blocks 6-7 are
        min-reduced straight from PSUM by the DVE.  The +BIG mask keeps
        same-class out of every min path; centers join via one small
        reduce.
  - Epilogue: sqrt on Scalar (table preloaded at t=0), relu on DVE,
    row-sum via a ones-matmul; per-core partials are averaged on the
    host (the "all-reduce mean").
"""

import numpy as np

N, D, NCTR, C = 8192, 128, 16, 64
NCORES = 8
RPC = N // NCORES          # rows per core = 1024
MT = RPC // 128            # m-tiles per core = 8
NCOL = N + NCTR            # 8208 columns (samples + centers)
NBLK = 8                   # full [128,1024] column blocks per m-tile
BIG = 4096.0
S = 64.0                   # sqrt(BIG)
MARGIN = 1.0
EPS = 1e-12
STRIP = 192                # strip margin (max class size it tolerates)
NCOPY = 7                  # blocks per m-tile copied to SBUF (rest: direct)

_CACHE = {}


def _strip_slices(m):
    """Per m-tile: 1-2 (block, lo, hi) slices covering the same-class strip
    [128*m - STRIP, 128*m + 128 + STRIP) in rotated column space."""
    lo = 128 * m - STRIP
    hi = 128 * m + 128 + STRIP
    out = []
    if lo < 0:
        out.append((7, 1024 + lo, 1024))
        lo = 0
    if hi <= 1024:
        out.append((0, lo, hi))
    else:
        out.append((0, lo, 1024))
        out.append((1, 0, hi - 1024))
    return out


def _build_program(warm_mms=10):
    from concourse import bacc, mybir, tile
    from concourse.bass import ts

    f32 = mybir.dt.float32
    bf16 = mybir.dt.bfloat16
    X = mybir.AxisListType.X
    Alu = mybir.AluOpType

    nc = bacc.Bacc(
        "TRN2", target_bir_lowering=False, debug=False, enable_asserts=False
    )

    rhs1_d = nc.dram_tensor("rhs1", [D, NCOL], bf16, kind="ExternalInput").ap()
    rhs2_d = nc.dram_tensor("rhs2", [C + 1, NCOL], bf16, kind="ExternalInput").ap()
    lhs1_d = nc.dram_tensor("lhs1", [D, RPC], bf16, kind="ExternalInput").ap()
    lhs2_d = nc.dram_tensor("lhs2", [C + 1, RPC], bf16, kind="ExternalInput").ap()
    sqi_d = nc.dram_tensor("sqi", [128, MT], f32, kind="ExternalInput").ap()
    out_d = nc.dram_tensor("out", [1, 1], f32, kind="ExternalOutput").ap()

    with tile.TileContext(nc) as tc:
        with (
            tc.tile_pool(name="per", bufs=1) as per,
            tc.tile_pool(name="cb", bufs=4) as cb,
            tc.tile_pool(name="tp", bufs=6) as tp,
        ):
            # ---- persistent SBUF tensors ----
            rhs1s = per.tile([D, NCOL], bf16, tag="rhs1s")
            rhs2s = per.tile([C + 1, NCOL], bf16, tag="rhs2s")
            lhs1s = per.tile([D, RPC], bf16, tag="lhs1s")
            lhs2s = per.tile([C + 1, RPC], bf16, tag="lhs2s")
            sqi = per.tile([128, MT], f32, tag="sqi")
            mins = per.tile([128, MT * 3], f32, tag="mins")
            maxs = per.tile([128, MT * 2], f32, tag="maxs")
            wgarb = per.tile([128, 512], bf16, tag="wgarb")
            onescol = per.tile([128, 1], f32, tag="onescol")
            sqjunk = per.tile([1, 1], f32, tag="sqjunk")
            outs = per.tile([1, 1], f32, tag="outs")
            pos2 = per.tile([128, MT], f32, tag="pos2")
            neg2 = per.tile([128, MT], f32, tag="neg2")
            apd = per.tile([128, MT], f32, tag="apd")
            andt = per.tile([128, MT], f32, tag="andt")
            rl = per.tile([128, MT], f32, tag="rl")
            rsum = per.tile([128, 1], f32, tag="rsum")

            # ---- input DMAs spread over 4 engine queues, earliest first ----
            # gpsimd leads with the warmup-garbage memset so the PE can
            # start immediately; vector leads with the small memsets.
            nc.gpsimd.memset(wgarb[:, :], 0.0)
            nc.vector.memset(onescol[:, :], 1.0)
            nc.vector.memset(maxs[:, :], -3.0e38)

            def r1(b):
                return (rhs1s[:, 1024 * b : 1024 * b + 1024],
                        rhs1_d[:, 1024 * b : 1024 * b + 1024])

            def r2(b):
                return (rhs2s[:, 1024 * b : 1024 * b + 1024],
                        rhs2_d[:, 1024 * b : 1024 * b + 1024])

            for o, i in [(lhs1s[:, :], lhs1_d[:, :]), r1(0), r2(2),
                         r1(2), r1(4), r1(6)]:
                nc.sync.dma_start(out=o, in_=i)
            for o, i in [(lhs2s[:, :], lhs2_d[:, :]), r1(1), r2(3),
                         r1(3), r1(5), r1(7)]:
                nc.gpsimd.dma_start(out=o, in_=i)
            for o, i in [r2(0), r2(1), r2(4), r2(5), r2(6), r2(7),
                         (rhs1s[:, N:], rhs1_d[:, N:]),
                         (rhs2s[:, N:], rhs2_d[:, N:]),
                         (sqi[:, :], sqi_d[:, :])]:
                nc.scalar.dma_start(out=o, in_=i)
            # preload the sqrt activation table while the sweep runs
            nc.scalar.sqrt(sqjunk[:, :], onescol[0:1, 0:1])

            # ---- main sweep ----
            with tc.tile_pool(name="pp", bufs=4, space="PSUM") as pp:
                if warm_mms:
                    # garbage matmuls: wake HAM out of the throttled clock
                    # while the real inputs are still in flight
                    wpt = pp.tile([128, 1024], f32, tag="ptile")
                    for _ in range(warm_mms):
                        nc.tensor.matmul(
                            wpt[:, 0:512], wgarb[:, 0:128], wgarb[:, :],
                            start=True, stop=True,
                        )

                for m in range(MT):
                    w1 = lhs1s[:, ts(m, 128)]
                    w2 = lhs2s[:, ts(m, 128)]
                    cts = {}          # block -> bf16 SBUF copy
                    tmins = []        # pairwise min tiles (bf16)
                    strips = _strip_slices(m)

                    def emit_consumers(b, pt):
                        for si, (blk, lo, hi) in enumerate(strips):
                            if blk == b:
                                nc.vector.tensor_reduce(
                                    maxs[:, 2 * m + si : 2 * m + si + 1],
                                    pt[:, lo:hi],
                                    X,
                                    Alu.max,
                                )
                        if b >= NCOPY:
                            # direct min-reduce from PSUM (fp32)
                            nc.vector.tensor_reduce(
                                mins[:, 3 * m + 1 : 3 * m + 2],
                                pt[:, :],
                                X,
                                Alu.min,
                            )
                            return
                        ct = cb.tile([128, 1024], bf16, tag="ct")
                        nc.scalar.copy(ct[:, :], pt[:, :])
                        cts[b] = ct
                        if b % 2 == 1:
                            tm = tp.tile([128, 1024], bf16, tag="tm")
                            nc.vector.tensor_tensor(
                                out=tm[:, :], in0=cts[b - 1][:, :],
                                in1=cts[b][:, :], op=Alu.min,
                            )
                            tmins.append(tm)
                            if b == 3:
                                ta = tp.tile([128, 1024], bf16, tag="tm")
                                nc.vector.tensor_tensor(
                                    out=ta[:, :], in0=tmins[0][:, :],
                                    in1=tmins[1][:, :], op=Alu.min,
                                )
                                tmins.append(ta)
                        if b == 6:
                            tb = tp.tile([128, 1024], bf16, tag="tm")
                            nc.vector.tensor_tensor(
                                out=tb[:, :], in0=tmins[1][:, :],
                                in1=cts[6][:, :], op=Alu.min,
                            )
                            tc = tp.tile([128, 1024], bf16, tag="tm")
                            nc.vector.tensor_tensor(
                                out=tc[:, :], in0=tmins[2][:, :],
                                in1=tb[:, :], op=Alu.min,
                            )
                            nc.vector.tensor_reduce(
                                mins[:, 3 * m : 3 * m + 1],
                                tc[:, :],
                                X,
                                Alu.min,
                            )

                    # paired blocks: pass1 over both, then pass2 over both
                    for bp in range(NBLK // 2):
                        b0, b1 = 2 * bp, 2 * bp + 1
                        pta = pp.tile([128, 1024], f32, tag="ptile")
                        ptb = pp.tile([128, 1024], f32, tag="ptile")
                        for pt, b in ((pta, b0), (ptb, b1)):
                            for h in range(2):
                                c0 = 1024 * b + 512 * h
                                nc.tensor.matmul(
                                    pt[:, h * 512 : h * 512 + 512],
                                    w1,
                                    rhs1s[:, c0 : c0 + 512],
                                    start=True,
                                    stop=False,
                                )
                        for pt, b in ((pta, b0), (ptb, b1)):
                            for h in range(2):
                                c0 = 1024 * b + 512 * h
                                nc.tensor.matmul(
                                    pt[:, h * 512 : h * 512 + 512],
                                    w2,
                                    rhs2s[:, c0 : c0 + 512],
                                    start=False,
                                    stop=True,
                                )
                        emit_consumers(b0, pta)
                        emit_consumers(b1, ptb)

                    # centers block (16 cols)
                    ptc = pp.tile([128, 1024], f32, tag="ptile")
                    nc.tensor.matmul(
                        ptc[:, 0:NCTR], w1, rhs1s[:, N:], start=True, stop=False
                    )
                    nc.tensor.matmul(
                        ptc[:, 0:NCTR], w2, rhs2s[:, N:], start=False, stop=True
                    )
                    nc.vector.tensor_reduce(
                        mins[:, 3 * m + 2 : 3 * m + 3], ptc[:, :NCTR], X, Alu.min
                    )

                # ---- epilogue (vectorized over the 8 m-tiles) ----
                posr = per.tile([128, MT], f32, tag="posr")
                negr = per.tile([128, MT], f32, tag="negr")
                nc.vector.tensor_reduce(
                    posr[:, :], maxs[:, :].rearrange("p (t s) -> p t s", s=2),
                    X, Alu.max,
                )
                nc.vector.tensor_reduce(
                    negr[:, :], mins[:, :].rearrange("p (t s) -> p t s", s=3),
                    X, Alu.min,
                )

                nc.vector.tensor_tensor(
                    out=pos2[:, :], in0=posr[:, :], in1=sqi[:, :], op=Alu.add
                )
                nc.vector.tensor_scalar(
                    out=pos2[:, :], in0=pos2[:, :], scalar1=BIG, scalar2=EPS,
                    op0=Alu.subtract, op1=Alu.max,
                )
                nc.scalar.sqrt(apd[:, :], pos2[:, :])

                nc.vector.tensor_tensor(
                    out=neg2[:, :], in0=negr[:, :], in1=sqi[:, :], op=Alu.add
                )
                nc.vector.tensor_scalar(
                    out=neg2[:, :], in0=neg2[:, :], scalar1=EPS, scalar2=None,
                    op0=Alu.max,
                )
                nc.scalar.sqrt(andt[:, :], neg2[:, :])

                nc.vector.tensor_tensor(
                    out=rl[:, :], in0=apd[:, :], in1=andt[:, :], op=Alu.subtract
                )
                nc.vector.tensor_scalar(
                    out=rl[:, :], in0=rl[:, :], scalar1=MARGIN, scalar2=0.0,
                    op0=Alu.add, op1=Alu.max,
                )
                nc.vector.tensor_reduce(rsum[:, :], rl[:, :], X, Alu.add)

                fin = pp.tile([128, 1024], f32, tag="ptile")
                nc.tensor.matmul(
                    fin[0:1, 0:1], onescol[:, :], rsum[:, :], start=True, stop=True
                )
                nc.scalar.copy(outs[:, :], fin[0:1, 0:1])
                nc.sync.dma_start(out=out_d[:, :], in_=outs[:, :])

    nc.compile()
    return nc


def _make_in_maps(inputs, targets, center):
    import ml_dtypes

    bf = ml_dtypes.bfloat16
    x = np.ascontiguousarray(np.asarray(inputs, dtype=np.float32))
    t = np.asarray(targets).astype(np.int64)
    c = np.ascontiguousarray(np.asarray(center, dtype=np.float32))

    # global class sort of rows == columns
    perm = np.argsort(t, kind="stable")
    xs = x[perm]
    ts_ = t[perm]
    sqs = (xs.astype(np.float64) ** 2).sum(1).astype(np.float32)

    cn = c / np.linalg.norm(c, axis=1, keepdims=True)

    xsT = np.ascontiguousarray(xs.T).astype(bf)             # [128, 8192]
    oh = (ts_[None, :] == np.arange(C)[:, None]).astype(np.float32) * S
    ohb = oh.astype(bf)                                      # [64, 8192]
    sq_row = sqs.astype(bf)                                  # [8192]

    in_maps = []
    for k in range(NCORES):
        sh = -RPC * k
        rows = slice(RPC * k, RPC * (k + 1))

        rhs1 = np.empty((D, NCOL), dtype=bf)
        rhs1[:, :N] = np.roll(xsT, sh, axis=1)
        rhs1[:, N:] = cn.T.astype(bf)

        rhs2 = np.zeros((C + 1, NCOL), dtype=bf)
        rhs2[:C, :N] = np.roll(ohb, sh, axis=1)
        rhs2[C, :N] = np.roll(sq_row, sh)
        rhs2[C, N:] = np.ones((NCTR,), dtype=bf)

        lhs1 = np.ascontiguousarray(-2.0 * xsT[:, rows].astype(np.float32)).astype(bf)
        lhs2 = np.concatenate(
            [ohb[:, rows], np.ones((1, RPC), dtype=bf)], axis=0
        )
        sqi = np.ascontiguousarray(
            sqs[rows].reshape(MT, 128).T
        )  # [128, MT]

        in_maps.append(
            {
                "rhs1": rhs1,
                "rhs2": rhs2,
                "lhs1": np.ascontiguousarray(lhs1),
                "lhs2": np.ascontiguousarray(lhs2),
                "sqi": sqi,
            }
        )
    return in_maps


def run(inputs, targets, center, trace=False, tmpdir=None):
    """Returns (loss_scalar, BassKernelResults)."""
    from concourse.bass_utils import run_bass_kernel_spmd

    if "nc" not in _CACHE:
        _CACHE["nc"] = _build_program()
    nc = _CACHE["nc"]
    in_maps = _make_in_maps(inputs, targets, center)
    res = run_bass_kernel_spmd(
        nc, in_maps, list(range(NCORES)), trace=trace, tmpdir=tmpdir
    )
    total = sum(float(r["out"][0, 0]) for r in res.results)
    loss = np.array(total / N, dtype=np.float32)
    return loss, res


def kernel(inputs, targets, center):
    loss, _ = run(inputs, targets, center, trace=False)
    return loss


# revision 19
# speedup vs baseline: 1.5078x; 1.1421x over previous
"""AugmentedTripletLoss kernel for 8 Trainium2 NeuronCores.

Strategy (data-parallel over rows, per sharding hint) — v4:
  - Rows AND columns are globally sorted by class (host-side, free: the
    loss is a mean over rows, permutation-invariant).  Core k takes
    sorted rows [1024k, 1024k+1024) and sees the 8192 columns ROTATED
    by 1024k, so its own rows sit at columns [0, 1024).  Same-class
    columns for m-tile m then live in the fixed strip
    [128m-STRIP, 128m+128+STRIP) mod 8192 (valid while every class has
    <= STRIP members; multinomial(8192, 64) gives ~128 +- 11, max ~165).
  - Per m-tile the [128, 8208] block of
        D(i,j) = dist2(i,j) - sq_i + BIG*mask(i,j)
    is built with two accumulated bf16 matmul passes per [128,1024]
    PSUM tile, with the two passes grouped across block pairs so
    LDWEIGHTS swaps half as often.  All operands are pre-baked on the
    host; input DMAs are spread over four engine queues so the first
    matmul starts ~2us in, and a burst of garbage-fed warmup matmuls
    un-throttles the PE clock (HAM) while they land.
  - Mining per m-tile:
      dist_ap^2: tensor_reduce(max) over the same-class strip read
        directly from PSUM in fp32 (+BIG selects same-class there).
      dist_an^2: blocks 0-5 are copied to SBUF as bf16 by the Scalar
        engine and min-combined by a DVE tensor_tensor tree (2x bf16
        mode) with GpSimd taking two interior tree ops; blocks 6-7 are
        min-reduced straight from PSUM by the DVE.  The +BIG mask keeps
        same-class out of every min path; centers join via one small
        reduce.
  - Epilogue: sqrt on Scalar (table preloaded at t=0), relu on DVE,
    row-sum via a ones-matmul; per-core partials are averaged on the
    host (the "all-reduce mean").
"""

import numpy as np

N, D, NCTR, C = 8192, 128, 16, 64
NCORES = 8
RPC = N // NCORES          # rows per core = 1024
MT = RPC // 128            # m-tiles per core = 8
NCOL = N + NCTR            # 8208 columns (samples + centers)
NBLK = 8                   # full [128,1024] column blocks per m-tile
BIG = 4096.0
S = 64.0                   # sqrt(BIG)
MARGIN = 1.0
EPS = 1e-12
STRIP = 192                # strip margin (max class size it tolerates)
NCOPY = 7                  # blocks per m-tile copied to SBUF (rest: direct)

_CACHE = {}


def _strip_slices(m):
    """Per m-tile: 1-2 (block, lo, hi) slices covering the same-class strip
    [128*m - STRIP, 128*m + 128 + STRIP) in rotated column space."""
    lo = 128 * m - STRIP
    hi = 128 * m + 128 + STRIP
    out = []
    if lo < 0:
        out.append((7, 1024 + lo, 1024))
        lo = 0
    if hi <= 1024:
        out.append((0, lo, hi))
    else:
        out.append((0, lo, 1024))
        out.append((1, 0, hi - 1024))
    return out


def _build_program(warm_mms=16):
    from concourse import bacc, mybir, tile
    from concourse.bass import ts

    f32 = mybir.dt.float32
    bf16 = mybir.dt.bfloat16
    X = mybir.AxisListType.X
    Alu = mybir.AluOpType

    nc = bacc.Bacc(
        "TRN2", target_bir_lowering=False, debug=False, enable_asserts=False
    )

    rhs1_d = nc.dram_tensor("rhs1", [D, NCOL], bf16, kind="ExternalInput").ap()
    rhs2_d = nc.dram_tensor("rhs2", [C + 1, NCOL], bf16, kind="ExternalInput").ap()
    lhs1_d = nc.dram_tensor("lhs1", [D, RPC], bf16, kind="ExternalInput").ap()
    lhs2_d = nc.dram_tensor("lhs2", [C + 1, RPC], bf16, kind="ExternalInput").ap()
    sqi_d = nc.dram_tensor("sqi", [128, MT], f32, kind="ExternalInput").ap()
    out_d = nc.dram_tensor("out", [1, 1], f32, kind="ExternalOutput").ap()

    with tile.TileContext(nc) as tc:
        with (
            tc.tile_pool(name="per", bufs=1) as per,
            tc.tile_pool(name="cb", bufs=4) as cb,
            tc.tile_pool(name="tp", bufs=6) as tp,
        ):
            # ---- persistent SBUF tensors ----
            rhs1s = per.tile([D, NCOL], bf16, tag="rhs1s")
            rhs2s = per.tile([C + 1, NCOL], bf16, tag="rhs2s")
            lhs1s = per.tile([D, RPC], bf16, tag="lhs1s")
            lhs2s = per.tile([C + 1, RPC], bf16, tag="lhs2s")
            sqi = per.tile([128, MT], f32, tag="sqi")
            mins = per.tile([128, MT * 4], f32, tag="mins")
            maxs = per.tile([128, MT * 2], f32, tag="maxs")
            wgarb = per.tile([128, 512], bf16, tag="wgarb")
            onescol = per.tile([128, 1], f32, tag="onescol")
            sqjunk = per.tile([1, 1], f32, tag="sqjunk")
            outs = per.tile([1, 1], f32, tag="outs")
            pos2 = per.tile([128, MT], f32, tag="pos2")
            neg2 = per.tile([128, MT], f32, tag="neg2")
            apd = per.tile([128, MT], f32, tag="apd")
            andt = per.tile([128, MT], f32, tag="andt")
            rl = per.tile([128, MT], f32, tag="rl")
            rsum = per.tile([128, 1], f32, tag="rsum")

            # ---- input DMAs spread over 4 engine queues, earliest first ----
            # gpsimd leads with the warmup-garbage memset so the PE can
            # start immediately; vector leads with the small memsets.
            nc.gpsimd.memset(wgarb[:, :], 0.0)
            nc.vector.memset(onescol[:, :], 1.0)
            nc.vector.memset(maxs[:, :], -3.0e38)
            nc.vector.memset(mins[:, :], 3.0e38)

            def r1(b):
                return (rhs1s[:, 1024 * b : 1024 * b + 1024],
                        rhs1_d[:, 1024 * b : 1024 * b + 1024])

            def r2(b):
                return (rhs2s[:, 1024 * b : 1024 * b + 1024],
                        rhs2_d[:, 1024 * b : 1024 * b + 1024])

            for o, i in [(lhs1s[:, :], lhs1_d[:, :]), r1(0), r1(3),
                         r1(6), r2(6), (sqi[:, :], sqi_d[:, :])]:
                nc.sync.dma_start(out=o, in_=i)
            for o, i in [(lhs2s[:, :], lhs2_d[:, :]), r1(1), r1(4),
                         r1(7), r2(7)]:
                nc.gpsimd.dma_start(out=o, in_=i)
            for o, i in [r2(0), r2(1), r1(2), r2(2), r2(3), r1(5),
                         r2(4), r2(5),
                         (rhs1s[:, N:], rhs1_d[:, N:]),
                         (rhs2s[:, N:], rhs2_d[:, N:])]:
                nc.scalar.dma_start(out=o, in_=i)
            # preload the sqrt activation table while the sweep runs
            nc.scalar.sqrt(sqjunk[:, :], onescol[0:1, 0:1])

            # ---- main sweep ----
            with tc.tile_pool(name="pp", bufs=4, space="PSUM") as pp:
                if warm_mms:
                    # garbage matmuls: wake HAM out of the throttled clock
                    # while the real inputs are still in flight
                    wpt = pp.tile([128, 1024], f32, tag="ptile")
                    for _ in range(warm_mms):
                        nc.tensor.matmul(
                            wpt[:, 0:512], wgarb[:, 0:128], wgarb[:, :],
                            start=True, stop=True,
                        )

                for m in range(MT):
                    w1 = lhs1s[:, ts(m, 128)]
                    w2 = lhs2s[:, ts(m, 128)]
                    ncopy = 5 if m == MT - 1 else 7
                    cts = {}          # block -> bf16 SBUF copy
                    tmins = []        # pairwise min tiles (bf16)
                    strips = _strip_slices(m)
                    # final tree tile: 1024 tree cols + 16 center cols
                    tcf = tp.tile([128, 1040], bf16, tag="tcf")

                    def emit_consumers(b, pt):
                        for si, (blk, lo, hi) in enumerate(strips):
                            if blk == b:
                                nc.vector.tensor_reduce(
                                    maxs[:, 2 * m + si : 2 * m + si + 1],
                                    pt[:, lo:hi],
                                    X,
                                    Alu.max,
                                )
                        if b >= ncopy:
                            # direct min-reduce from PSUM (fp32)
                            slot = 4 * m + 1 + (b - ncopy)
                            nc.vector.tensor_reduce(
                                mins[:, slot : slot + 1],
                                pt[:, :],
                                X,
                                Alu.min,
                            )
                            return
                        ct = cb.tile([128, 1024], bf16, tag="ct")
                        nc.scalar.copy(ct[:, :], pt[:, :])
                        cts[b] = ct
                        if b % 2 == 1:
                            tm = tp.tile([128, 1024], bf16, tag="tm")
                            nc.vector.tensor_tensor(
                                out=tm[:, :], in0=cts[b - 1][:, :],
                                in1=cts[b][:, :], op=Alu.min,
                            )
                            tmins.append(tm)
                            if b == 3:
                                ta = tp.tile([128, 1024], bf16, tag="tm")
                                nc.vector.tensor_tensor(
                                    out=ta[:, :], in0=tmins[0][:, :],
                                    in1=tmins[1][:, :], op=Alu.min,
                                )
                                tmins.append(ta)
                        if ncopy == 5 and b == 4:
                            # short tree: min(ta, c4) -> tcf
                            nc.vector.tensor_tensor(
                                out=tcf[:, 0:1024], in0=tmins[2][:, :],
                                in1=cts[4][:, :], op=Alu.min,
                            )
                        if ncopy == 7 and b == 6:
                            tb = tp.tile([128, 1024], bf16, tag="tm")
                            nc.vector.tensor_tensor(
                                out=tb[:, :], in0=tmins[1][:, :],
                                in1=cts[6][:, :], op=Alu.min,
                            )
                            nc.vector.tensor_tensor(
                                out=tcf[:, 0:1024], in0=tmins[2][:, :],
                                in1=tb[:, :], op=Alu.min,
                            )

                    # paired blocks: pass1 over both, then pass2 over both
                    for bp in range(NBLK // 2):
                        b0, b1 = 2 * bp, 2 * bp + 1
                        pta = pp.tile([128, 1024], f32, tag="ptile")
                        ptb = pp.tile([128, 1024], f32, tag="ptile")
                        for pt, b in ((pta, b0), (ptb, b1)):
                            for h in range(2):
                                c0 = 1024 * b + 512 * h
                                nc.tensor.matmul(
                                    pt[:, h * 512 : h * 512 + 512],
                                    w1,
                                    rhs1s[:, c0 : c0 + 512],
                                    start=True,
                                    stop=False,
                                )
                        for pt, b in ((pta, b0), (ptb, b1)):
                            for h in range(2):
                                c0 = 1024 * b + 512 * h
                                nc.tensor.matmul(
                                    pt[:, h * 512 : h * 512 + 512],
                                    w2,
                                    rhs2s[:, c0 : c0 + 512],
                                    start=False,
                                    stop=True,
                                )
                        emit_consumers(b0, pta)
                        emit_consumers(b1, ptb)

                    # centers block (16 cols)
                    ptc = pp.tile([128, 1024], f32, tag="ptile")
                    nc.tensor.matmul(
                        ptc[:, 0:NCTR], w1, rhs1s[:, N:], start=True, stop=False
                    )
                    nc.tensor.matmul(
                        ptc[:, 0:NCTR], w2, rhs2s[:, N:], start=False, stop=True
                    )
                    nc.scalar.copy(tcf[:, 1024:1040], ptc[:, :NCTR])
                    nc.vector.tensor_reduce(
                        mins[:, 4 * m : 4 * m + 1], tcf[:, :], X, Alu.min
                    )

                # ---- epilogue (vectorized over the 8 m-tiles) ----
                posr = per.tile([128, MT], f32, tag="posr")
                negr = per.tile([128, MT], f32, tag="negr")
                nc.vector.tensor_reduce(
                    posr[:, :], maxs[:, :].rearrange("p (t s) -> p t s", s=2),
                    X, Alu.max,
                )
                nc.vector.tensor_reduce(
                    negr[:, :], mins[:, :].rearrange("p (t s) -> p t s", s=4),
                    X, Alu.min,
                )

                nc.vector.tensor_tensor(
                    out=pos2[:, :], in0=posr[:, :], in1=sqi[:, :], op=Alu.add
                )
                nc.vector.tensor_scalar(
                    out=pos2[:, :], in0=pos2[:, :], scalar1=BIG, scalar2=EPS,
                    op0=Alu.subtract, op1=Alu.max,
                )
                nc.scalar.sqrt(apd[:, :], pos2[:, :])

                nc.vector.tensor_tensor(
                    out=neg2[:, :], in0=negr[:, :], in1=sqi[:, :], op=Alu.add
                )
                nc.vector.tensor_scalar(
                    out=neg2[:, :], in0=neg2[:, :], scalar1=EPS, scalar2=None,
                    op0=Alu.max,
                )
                nc.scalar.sqrt(andt[:, :], neg2[:, :])

                nc.vector.tensor_tensor(
                    out=rl[:, :], in0=apd[:, :], in1=andt[:, :], op=Alu.subtract
                )
                nc.vector.tensor_scalar(
                    out=rl[:, :], in0=rl[:, :], scalar1=MARGIN, scalar2=0.0,
                    op0=Alu.add, op1=Alu.max,
                )
                nc.vector.tensor_reduce(rsum[:, :], rl[:, :], X, Alu.add)

                fin = pp.tile([128, 1024], f32, tag="ptile")
                nc.tensor.matmul(
                    fin[0:1, 0:1], onescol[:, :], rsum[:, :], start=True, stop=True
                )
                nc.scalar.copy(outs[:, :], fin[0:1, 0:1])
                nc.sync.dma_start(out=out_d[:, :], in_=outs[:, :])

    nc.compile()
    return nc


def _make_in_maps(inputs, targets, center):
    import ml_dtypes

    bf = ml_dtypes.bfloat16
    x = np.ascontiguousarray(np.asarray(inputs, dtype=np.float32))
    t = np.asarray(targets).astype(np.int64)
    c = np.ascontiguousarray(np.asarray(center, dtype=np.float32))

    # global class sort of rows == columns
    perm = np.argsort(t, kind="stable")
    xs = x[perm]
    ts_ = t[perm]
    sqs = (xs.astype(np.float64) ** 2).sum(1).astype(np.float32)

    cn = c / np.linalg.norm(c, axis=1, keepdims=True)

    xsT = np.ascontiguousarray(xs.T).astype(bf)             # [128, 8192]
    oh = (ts_[None, :] == np.arange(C)[:, None]).astype(np.float32) * S
    ohb = oh.astype(bf)                                      # [64, 8192]
    sq_row = sqs.astype(bf)                                  # [8192]

    in_maps = []
    for k in range(NCORES):
        sh = -RPC * k
        rows = slice(RPC * k, RPC * (k + 1))

        rhs1 = np.empty((D, NCOL), dtype=bf)
        rhs1[:, :N] = np.roll(xsT, sh, axis=1)
        rhs1[:, N:] = cn.T.astype(bf)

        rhs2 = np.zeros((C + 1, NCOL), dtype=bf)
        rhs2[:C, :N] = np.roll(ohb, sh, axis=1)
        rhs2[C, :N] = np.roll(sq_row, sh)
        rhs2[C, N:] = np.ones((NCTR,), dtype=bf)

        lhs1 = np.ascontiguousarray(-2.0 * xsT[:, rows].astype(np.float32)).astype(bf)
        lhs2 = np.concatenate(
            [ohb[:, rows], np.ones((1, RPC), dtype=bf)], axis=0
        )
        sqi = np.ascontiguousarray(
            sqs[rows].reshape(MT, 128).T
        )  # [128, MT]

        in_maps.append(
            {
                "rhs1": rhs1,
                "rhs2": rhs2,
                "lhs1": np.ascontiguousarray(lhs1),
                "lhs2": np.ascontiguousarray(lhs2),
                "sqi": sqi,
            }
        )
    return in_maps


def run(inputs, targets, center, trace=False, tmpdir=None):
    """Returns (loss_scalar, BassKernelResults)."""
    from concourse.bass_utils import run_bass_kernel_spmd

    if "nc" not in _CACHE:
        _CACHE["nc"] = _build_program()
    nc = _CACHE["nc"]
    in_maps = _make_in_maps(inputs, targets, center)
    res = run_bass_kernel_spmd(
        nc, in_maps, list(range(NCORES)), trace=trace, tmpdir=tmpdir
    )
    total = sum(float(r["out"][0, 0]) for r in res.results)
    loss = np.array(total / N, dtype=np.float32)
    return loss, res


def kernel(inputs, targets, center):
    loss, _ = run(inputs, targets, center, trace=False)
    return loss


# revision 20
# speedup vs baseline: 1.5939x; 1.0571x over previous
"""AugmentedTripletLoss kernel for 8 Trainium2 NeuronCores.

Strategy (data-parallel over rows, per sharding hint) — v4:
  - Rows AND columns are globally sorted by class (host-side, free: the
    loss is a mean over rows, permutation-invariant).  Core k takes
    sorted rows [1024k, 1024k+1024) and sees the 8192 columns ROTATED
    by 1024k, so its own rows sit at columns [0, 1024).  Same-class
    columns for m-tile m then live in the fixed strip
    [128m-STRIP, 128m+128+STRIP) mod 8192 (valid while every class has
    <= STRIP members; multinomial(8192, 64) gives ~128 +- 11, max ~165).
  - Per m-tile the [128, 8208] block of
        D(i,j) = dist2(i,j) - sq_i + BIG*mask(i,j)
    is built with two accumulated bf16 matmul passes per [128,1024]
    PSUM tile, with the two passes grouped across block pairs so
    LDWEIGHTS swaps half as often.  All operands are pre-baked on the
    host; input DMAs are spread over four engine queues so the first
    matmul starts ~2us in, and a burst of garbage-fed warmup matmuls
    un-throttles the PE clock (HAM) while they land.
  - Mining per m-tile:
      dist_ap^2: tensor_reduce(max) over the same-class strip read
        directly from PSUM in fp32 (+BIG selects same-class there).
      dist_an^2: blocks 0-5 are copied to SBUF as bf16 by the Scalar
        engine and min-combined by a DVE tensor_tensor tree (2x bf16
        mode) with GpSimd taking two interior tree ops; blocks 6-7 are
        min-reduced straight from PSUM by the DVE.  The +BIG mask keeps
        same-class out of every min path; centers join via one small
        reduce.
  - Epilogue: sqrt on Scalar (table preloaded at t=0), relu on DVE,
    row-sum via a ones-matmul; per-core partials are averaged on the
    host (the "all-reduce mean").
"""

import numpy as np

N, D, NCTR, C = 8192, 128, 16, 64
NCORES = 8
RPC = N // NCORES          # rows per core = 1024
MT = RPC // 128            # m-tiles per core = 8
NCOL = N + NCTR            # 8208 columns (samples + centers)
NBLK = 8                   # full [128,1024] column blocks per m-tile
BIG = 4096.0
S = 64.0                   # sqrt(BIG)
MARGIN = 1.0
EPS = 1e-12
STRIP = 192                # strip margin (max class size it tolerates)
NCOPY = 7                  # blocks per m-tile copied to SBUF (rest: direct)

_CACHE = {}


def _strip_slices(m):
    """Per m-tile: 1-2 (block, lo, hi) slices covering the same-class strip
    [128*m - STRIP, 128*m + 128 + STRIP) in rotated column space."""
    lo = 128 * m - STRIP
    hi = 128 * m + 128 + STRIP
    out = []
    if lo < 0:
        out.append((7, 1024 + lo, 1024))
        lo = 0
    if hi <= 1024:
        out.append((0, lo, hi))
    else:
        out.append((0, lo, 1024))
        out.append((1, 0, hi - 1024))
    return out


def _build_program(warm_mms=16):
    from concourse import bacc, mybir, tile
    from concourse.bass import ts

    f32 = mybir.dt.float32
    bf16 = mybir.dt.bfloat16
    X = mybir.AxisListType.X
    Alu = mybir.AluOpType

    nc = bacc.Bacc(
        "TRN2", target_bir_lowering=False, debug=False, enable_asserts=False
    )

    rhs1_d = nc.dram_tensor("rhs1", [D, NCOL], bf16, kind="ExternalInput").ap()
    rhs2_d = nc.dram_tensor("rhs2", [C + 1, NCOL], bf16, kind="ExternalInput").ap()
    lhs1_d = nc.dram_tensor("lhs1", [D, RPC], bf16, kind="ExternalInput").ap()
    lhs2_d = nc.dram_tensor("lhs2", [C + 1, RPC], bf16, kind="ExternalInput").ap()
    sqi_d = nc.dram_tensor("sqi", [128, MT], f32, kind="ExternalInput").ap()
    out_d = nc.dram_tensor("out", [1, 1], f32, kind="ExternalOutput").ap()

    with tile.TileContext(nc) as tc:
        with (
            tc.tile_pool(name="per", bufs=1) as per,
            tc.tile_pool(name="cb", bufs=4) as cb,
            tc.tile_pool(name="tp", bufs=6) as tp,
        ):
            # ---- persistent SBUF tensors ----
            rhs1s = per.tile([D, NCOL], bf16, tag="rhs1s")
            rhs2s = per.tile([C + 1, NCOL], bf16, tag="rhs2s")
            lhs1s = per.tile([D, RPC], bf16, tag="lhs1s")
            lhs2s = per.tile([C + 1, RPC], bf16, tag="lhs2s")
            sqi = per.tile([128, MT], f32, tag="sqi")
            mins = per.tile([128, MT * 4], f32, tag="mins")
            maxs = per.tile([128, MT * 2], f32, tag="maxs")
            wgarb = per.tile([128, 512], bf16, tag="wgarb")
            onescol = per.tile([128, 1], f32, tag="onescol")
            sqjunk = per.tile([1, 1], f32, tag="sqjunk")
            outs = per.tile([1, 1], f32, tag="outs")
            pos2 = per.tile([128, MT], f32, tag="pos2")
            neg2 = per.tile([128, MT], f32, tag="neg2")
            apd = per.tile([128, MT], f32, tag="apd")
            andt = per.tile([128, MT], f32, tag="andt")
            rl = per.tile([128, MT], f32, tag="rl")
            rsum = per.tile([128, 1], f32, tag="rsum")

            # ---- input DMAs spread over 4 engine queues, earliest first ----
            # gpsimd leads with the warmup-garbage memset so the PE can
            # start immediately; vector leads with the small memsets.
            nc.gpsimd.memset(wgarb[:, :], 0.0)
            nc.vector.memset(onescol[:, :], 1.0)
            nc.vector.memset(maxs[:, :], -3.0e38)
            nc.vector.memset(mins[:, :], 3.0e38)

            def r1(b):
                return (rhs1s[:, 1024 * b : 1024 * b + 1024],
                        rhs1_d[:, 1024 * b : 1024 * b + 1024])

            def r2(b):
                return (rhs2s[:, 1024 * b : 1024 * b + 1024],
                        rhs2_d[:, 1024 * b : 1024 * b + 1024])

            for o, i in [(lhs1s[:, :], lhs1_d[:, :]), r1(0), r1(2),
                         r2(3), r1(5), r2(6),
                         (rhs1s[:, N:], rhs1_d[:, N:]),
                         (sqi[:, :], sqi_d[:, :])]:
                nc.sync.dma_start(out=o, in_=i)
            for o, i in [(lhs2s[:, :], lhs2_d[:, :]), r2(0), r2(2),
                         r1(4), r2(5), r1(7),
                         (rhs2s[:, N:], rhs2_d[:, N:])]:
                nc.gpsimd.dma_start(out=o, in_=i)
            for o, i in [r1(1), r2(1), r1(3), r2(4), r1(6), r2(7)]:
                nc.scalar.dma_start(out=o, in_=i)
            # preload the sqrt activation table while the sweep runs
            nc.scalar.sqrt(sqjunk[:, :], onescol[0:1, 0:1])

            # ---- main sweep ----
            with tc.tile_pool(name="pp", bufs=4, space="PSUM") as pp:
                if warm_mms:
                    # garbage matmuls: wake HAM out of the throttled clock
                    # while the real inputs are still in flight
                    wpt = pp.tile([128, 1024], f32, tag="ptile")
                    for _ in range(warm_mms):
                        nc.tensor.matmul(
                            wpt[:, 0:512], wgarb[:, 0:128], wgarb[:, :],
                            start=True, stop=True,
                        )

                for m in range(MT):
                    w1 = lhs1s[:, ts(m, 128)]
                    w2 = lhs2s[:, ts(m, 128)]
                    ncopy = 7
                    cts = {}          # block -> bf16 SBUF copy
                    tmins = []        # pairwise min tiles (bf16)
                    strips = _strip_slices(m)
                    # final tree tile: 1024 tree cols + 16 center cols
                    tcf = tp.tile([128, 1040], bf16, tag="tcf")

                    def emit_consumers(b, pt):
                        for si, (blk, lo, hi) in enumerate(strips):
                            if blk == b:
                                nc.vector.tensor_reduce(
                                    maxs[:, 2 * m + si : 2 * m + si + 1],
                                    pt[:, lo:hi],
                                    X,
                                    Alu.max,
                                )
                        if b >= ncopy:
                            # direct min-reduce from PSUM (fp32)
                            slot = 4 * m + 1 + (b - ncopy)
                            nc.vector.tensor_reduce(
                                mins[:, slot : slot + 1],
                                pt[:, :],
                                X,
                                Alu.min,
                            )
                            return
                        ct = cb.tile([128, 1024], bf16, tag="ct")
                        nc.scalar.copy(ct[:, :], pt[:, :])
                        cts[b] = ct
                        if b % 2 == 1:
                            tm = tp.tile([128, 1024], bf16, tag="tm")
                            nc.vector.tensor_tensor(
                                out=tm[:, :], in0=cts[b - 1][:, :],
                                in1=cts[b][:, :], op=Alu.min,
                            )
                            tmins.append(tm)
                            if b == 3:
                                ta = tp.tile([128, 1024], bf16, tag="tm")
                                nc.vector.tensor_tensor(
                                    out=ta[:, :], in0=tmins[0][:, :],
                                    in1=tmins[1][:, :], op=Alu.min,
                                )
                                tmins.append(ta)
                        if ncopy == 5 and b == 4:
                            # short tree: min(ta, c4) -> tcf
                            nc.vector.tensor_tensor(
                                out=tcf[:, 0:1024], in0=tmins[2][:, :],
                                in1=cts[4][:, :], op=Alu.min,
                            )
                        if ncopy == 7 and b == 6:
                            tb = tp.tile([128, 1024], bf16, tag="tm")
                            nc.vector.tensor_tensor(
                                out=tb[:, :], in0=tmins[1][:, :],
                                in1=cts[6][:, :], op=Alu.min,
                            )
                            nc.vector.tensor_tensor(
                                out=tcf[:, 0:1024], in0=tmins[2][:, :],
                                in1=tb[:, :], op=Alu.min,
                            )

                    # paired blocks: pass1 over both, then pass2 over both
                    for bp in range(NBLK // 2):
                        b0, b1 = 2 * bp, 2 * bp + 1
                        pta = pp.tile([128, 1024], f32, tag="ptile")
                        ptb = pp.tile([128, 1024], f32, tag="ptile")
                        for pt, b in ((pta, b0), (ptb, b1)):
                            for h in range(2):
                                c0 = 1024 * b + 512 * h
                                nc.tensor.matmul(
                                    pt[:, h * 512 : h * 512 + 512],
                                    w1,
                                    rhs1s[:, c0 : c0 + 512],
                                    start=True,
                                    stop=False,
                                )
                        for pt, b in ((pta, b0), (ptb, b1)):
                            for h in range(2):
                                c0 = 1024 * b + 512 * h
                                nc.tensor.matmul(
                                    pt[:, h * 512 : h * 512 + 512],
                                    w2,
                                    rhs2s[:, c0 : c0 + 512],
                                    start=False,
                                    stop=True,
                                )
                        emit_consumers(b0, pta)
                        emit_consumers(b1, ptb)

                    # centers block (16 cols)
                    ptc = pp.tile([128, 1024], f32, tag="ptile")
                    nc.tensor.matmul(
                        ptc[:, 0:NCTR], w1, rhs1s[:, N:], start=True, stop=False
                    )
                    nc.tensor.matmul(
                        ptc[:, 0:NCTR], w2, rhs2s[:, N:], start=False, stop=True
                    )
                    nc.scalar.copy(tcf[:, 1024:1040], ptc[:, :NCTR])
                    nc.vector.tensor_reduce(
                        mins[:, 4 * m : 4 * m + 1], tcf[:, :], X, Alu.min
                    )

                # ---- epilogue (vectorized over the 8 m-tiles) ----
                posr = per.tile([128, MT], f32, tag="posr")
                negr = per.tile([128, MT], f32, tag="negr")
                nc.vector.tensor_reduce(
                    posr[:, :], maxs[:, :].rearrange("p (t s) -> p t s", s=2),
                    X, Alu.max,
                )
                nc.vector.tensor_reduce(
                    negr[:, :], mins[:, :].rearrange("p (t s) -> p t s", s=4),
                    X, Alu.min,
                )

                nc.vector.tensor_tensor(
                    out=pos2[:, :], in0=posr[:, :], in1=sqi[:, :], op=Alu.add
                )
                nc.vector.tensor_scalar(
                    out=pos2[:, :], in0=pos2[:, :], scalar1=BIG, scalar2=EPS,
                    op0=Alu.subtract, op1=Alu.max,
                )
                nc.scalar.sqrt(apd[:, :], pos2[:, :])

                nc.vector.tensor_tensor(
                    out=neg2[:, :], in0=negr[:, :], in1=sqi[:, :], op=Alu.add
                )
                nc.vector.tensor_scalar(
                    out=neg2[:, :], in0=neg2[:, :], scalar1=EPS, scalar2=None,
                    op0=Alu.max,
                )
                nc.scalar.sqrt(andt[:, :], neg2[:, :])

                nc.vector.tensor_tensor(
                    out=rl[:, :], in0=apd[:, :], in1=andt[:, :], op=Alu.subtract
                )
                nc.vector.tensor_scalar(
                    out=rl[:, :], in0=rl[:, :], scalar1=MARGIN, scalar2=0.0,
                    op0=Alu.add, op1=Alu.max,
                )
                nc.vector.tensor_reduce(rsum[:, :], rl[:, :], X, Alu.add)

                fin = pp.tile([128, 1024], f32, tag="ptile")
                nc.tensor.matmul(
                    fin[0:1, 0:1], onescol[:, :], rsum[:, :], start=True, stop=True
                )
                nc.scalar.copy(outs[:, :], fin[0:1, 0:1])
                nc.sync.dma_start(out=out_d[:, :], in_=outs[:, :])

    nc.compile()
    return nc


def _make_in_maps(inputs, targets, center):
    import ml_dtypes

    bf = ml_dtypes.bfloat16
    x = np.ascontiguousarray(np.asarray(inputs, dtype=np.float32))
    t = np.asarray(targets).astype(np.int64)
    c = np.ascontiguousarray(np.asarray(center, dtype=np.float32))

    # global class sort of rows == columns
    perm = np.argsort(t, kind="stable")
    xs = x[perm]
    ts_ = t[perm]
    sqs = (xs.astype(np.float64) ** 2).sum(1).astype(np.float32)

    cn = c / np.linalg.norm(c, axis=1, keepdims=True)

    xsT = np.ascontiguousarray(xs.T).astype(bf)             # [128, 8192]
    oh = (ts_[None, :] == np.arange(C)[:, None]).astype(np.float32) * S
    ohb = oh.astype(bf)                                      # [64, 8192]
    sq_row = sqs.astype(bf)                                  # [8192]

    in_maps = []
    for k in range(NCORES):
        sh = -RPC * k
        rows = slice(RPC * k, RPC * (k + 1))

        rhs1 = np.empty((D, NCOL), dtype=bf)
        rhs1[:, :N] = np.roll(xsT, sh, axis=1)
        rhs1[:, N:] = cn.T.astype(bf)

        rhs2 = np.zeros((C + 1, NCOL), dtype=bf)
        rhs2[:C, :N] = np.roll(ohb, sh, axis=1)
        rhs2[C, :N] = np.roll(sq_row, sh)
        rhs2[C, N:] = np.ones((NCTR,), dtype=bf)

        lhs1 = np.ascontiguousarray(-2.0 * xsT[:, rows].astype(np.float32)).astype(bf)
        lhs2 = np.concatenate(
            [ohb[:, rows], np.ones((1, RPC), dtype=bf)], axis=0
        )
        sqi = np.ascontiguousarray(
            sqs[rows].reshape(MT, 128).T
        )  # [128, MT]

        in_maps.append(
            {
                "rhs1": rhs1,
                "rhs2": rhs2,
                "lhs1": np.ascontiguousarray(lhs1),
                "lhs2": np.ascontiguousarray(lhs2),
                "sqi": sqi,
            }
        )
    return in_maps


def run(inputs, targets, center, trace=False, tmpdir=None):
    """Returns (loss_scalar, BassKernelResults)."""
    from concourse.bass_utils import run_bass_kernel_spmd

    if "nc" not in _CACHE:
        _CACHE["nc"] = _build_program()
    nc = _CACHE["nc"]
    in_maps = _make_in_maps(inputs, targets, center)
    res = run_bass_kernel_spmd(
        nc, in_maps, list(range(NCORES)), trace=trace, tmpdir=tmpdir
    )
    total = sum(float(r["out"][0, 0]) for r in res.results)
    loss = np.array(total / N, dtype=np.float32)
    return loss, res


def kernel(inputs, targets, center):
    loss, _ = run(inputs, targets, center, trace=False)
    return loss
